# revision 27
# baseline (speedup 1.0000x reference)
"""Bilinear RoI pooling (7x7) on 8 Trainium2 NeuronCores.

Strategy (data-parallel over RoIs, per the sharding hint):
  - B=1024 boxes split into 8 slices of 128; the feature map is replicated.
  - Host builds TWO pair-interleaved copies of the zero-padded feature map
    (132x132 with a 2-px zero border), same slot numbering: each slot is one
    (row-pair, x) column, so the 2x2 bilinear patch of any corner is 2
    adjacent slots = one contiguous gather window.
      * fgat16: fp16, 2KB slots (one 4KB window per point)
      * fgat8:  int8 quantized per-PIXEL (the dequant scale is folded into
        that corner's bilinear weight on the host -> no device dequant),
        1KB slots (one 2KB window per point)
  - 28 of the 49 grid points gather int8, 21 gather fp16.  The mix balances
    the HBM gather traffic (~64us) against the ACT-engine int8-multiply rate
    (3 muls/point on the 28 int8 points ~ 58us) and keeps the DVE duty-cycle
    low enough that SWDGE descriptor generation (which DVE 2-port mode locks
    out of SBUF) does not starve.
  - Gathers are issued as dma_gather ops (7 grid points x 128 boxes = 896
    indices each): SWDGE emits one descriptor per index, amortizing per-op
    overhead ~128x vs per-(box,point) indirect DMAs.  Gather indices are
    int16, wrapped into 16 partitions and REPLICATED in all 8 16-partition
    groups (each GPSIMD Q7 core reads its own copy).
  - Blend uses ONLY tensor_scalar and tensor_tensor DVE ops (both ~180ns for
    [128,512] fp16) -- scalar_tensor_tensor measures ~650ns and in-place
    accumulation chains serialize at op latency, so both are avoided; ops are
    emitted level-major across each chunk's 7 points.
  - Out-of-bounds corners read zero border rows/cols (clamped indices), so no
    in-bounds masking is needed.

Device layout: partition = box (128/core); 49 grid points along free dim.
Output is fp16 on device; host casts to f32.
"""

import numpy as np

P = 128          # boxes per core == SBUF partitions
C = 512          # channels
NPT = 49         # 7*7 grid points
WP2 = 132        # padded width (2 zero cols each side)
HP2 = 132        # padded height (2 zero rows top, 2 bottom)
NBLK_E = 66      # even row-pairs (rows 0..131)
NBLK_O = 65      # odd row-pairs (rows 1..130)
NSLOT_E = NBLK_E * WP2
NSLOT = (NBLK_E + NBLK_O) * WP2   # 17292 slots
NCORES = 8
CHUNK = 7        # grid points per dma_gather op
NCHUNK = NPT // CHUNK
NCH8 = 4         # int8 chunks (points 0..27)
NPT8 = NCH8 * CHUNK
IDXCOLS = CHUNK * P // 16         # idx cols per chunk in the wrapped layout

_STATE = {}


def _build_nc(repeats=1, g8bufs=2, g16bufs=2, abufs=3, tbufs=2, mode="full",
              nch8=NCH8):
    """mode: 'full' | 'noblend' (gather+copy+store) | 'nodma' (blend from
    const tiles, no gathers)."""
    import concourse.bass as bass
    import concourse.bacc as bacc
    import concourse.tile as tile
    from concourse import mybir

    F32 = mybir.dt.float32
    F16 = mybir.dt.float16
    I16 = mybir.dt.int16
    I8 = mybir.dt.int8
    Alu = mybir.AluOpType

    nc = bacc.Bacc()
    fgat8 = nc.declare_dram_parameter("fgat8", [NSLOT, 2 * C], I8, isOutput=False)
    fgat16 = nc.declare_dram_parameter("fgat16", [NSLOT, 2 * C], F16, isOutput=False)
    gidx = nc.declare_dram_parameter(
        "gidx", [P, NCHUNK * IDXCOLS], I16, isOutput=False)
    # wts = [w00 | w10 | w01 | w11] blocks of NPT, f32 (scale-folded for the
    # int8 points); gathered corner order per point is [A0, B0, A1, B1]
    wts = nc.declare_dram_parameter("wts", [P, 4 * NPT], F32, isOutput=False)
    out = nc.declare_dram_parameter("out", [P, NPT * C], F16, isOutput=True)

    with tile.TileContext(nc) as tc:
        with (
            tc.tile_pool(name="const", bufs=1) as cpool,
            tc.tile_pool(name="apool", bufs=abufs) as apool,
            tc.tile_pool(name="g8pool", bufs=g8bufs) as g8pool,
            tc.tile_pool(name="g16pool", bufs=g16bufs) as g16pool,
            tc.tile_pool(name="tpool", bufs=tbufs) as tpool,
        ):
            idx = cpool.tile([P, NCHUNK * IDXCOLS], I16)
            nc.sync.dma_start(out=idx[:], in_=gidx[:])
            w = cpool.tile([P, 4 * NPT], F32)
            nc.scalar.dma_start(out=w[:], in_=wts[:])
            w00 = w[:, 0 * NPT:1 * NPT]
            w10 = w[:, 1 * NPT:2 * NPT]
            w01 = w[:, 2 * NPT:3 * NPT]
            w11 = w[:, 3 * NPT:4 * NPT]

            gc8 = gc16 = None
            if mode == "nodma":
                gc8 = cpool.tile([P, CHUNK * 4 * C], I8, tag="gc8")
                nc.vector.memset(gc8[:], 3)
                gc16 = cpool.tile([P, CHUNK * 4 * C], F16, tag="gc16")
                nc.vector.memset(gc16[:], 1.0)

            # interleave int8/fp16 chunks so ACT (int8 muls) and DVE (fp16
            # chains) fill in parallel
            order = []
            a, b = 0, nch8
            while a < nch8 or b < NCHUNK:
                if a < nch8:
                    order.append(a); a += 1
                if b < NCHUNK:
                    order.append(b); b += 1

            def gather(ch, is8):
                pool, tabl, dt = (
                    (g8pool, fgat8, I8) if is8 else (g16pool, fgat16, F16))
                g = pool.tile([P, CHUNK * 4 * C], dt, tag="g")
                src = tabl[:]
                src.ap[0] = [2 * C, NSLOT - 1]
                src.ap[1] = [1, 4 * C]
                g3 = g[:].rearrange("p (a b) -> p a b", a=CHUNK, b=4 * C)
                nc.gpsimd.dma_gather(
                    out_ap=g3,
                    in_ap=src,
                    idxs_ap=idx[:, ch * IDXCOLS:(ch + 1) * IDXCOLS],
                    num_idxs=CHUNK * P,
                    num_idxs_reg=CHUNK * P,
                    elem_size=4 * C,
                    elem_step=2 * C,
                )
                return g

            for rep in range(repeats):
                for ch in order:
                    is8 = ch < nch8
                    if mode == "nodma":
                        g = gc8 if is8 else gc16
                    else:
                        g = gather(ch, is8)
                    afat = apool.tile([P, CHUNK * C], F16, tag="afat")

                    def A0(k):
                        return g[:, (4 * k + 0) * C:(4 * k + 1) * C]

                    def B0(k):
                        return g[:, (4 * k + 1) * C:(4 * k + 2) * C]

                    def A1(k):
                        return g[:, (4 * k + 2) * C:(4 * k + 3) * C]

                    def B1(k):
                        return g[:, (4 * k + 3) * C:(4 * k + 4) * C]

                    def ac(k):
                        return afat[:, k * C:(k + 1) * C]

                    if mode == "noblend":
                        for k in range(CHUNK):
                            nc.vector.tensor_copy(out=ac(k), in_=A0(k))
                        nc.sync.dma_start(
                            out=out[:, ch * CHUNK * C:(ch + 1) * CHUNK * C],
                            in_=afat[:, 0:CHUNK * C])
                        continue
                    # ts/tt only, level-major across the chunk's 7 points
                    if is8:
                        # ACT: 3 int8 muls; DVE: 1 int8 ts + 3 tt
                        ut = tpool.tile([P, 3 * CHUNK * C], F16, tag="ut")

                        def u(j, k):
                            return ut[:, (j * CHUNK + k) * C:
                                      (j * CHUNK + k + 1) * C]
                        for k in range(CHUNK):
                            t = ch * CHUNK + k
                            nc.scalar.mul(u(0, k), B0(k), w10[:, t:t + 1])
                        for k in range(CHUNK):
                            t = ch * CHUNK + k
                            nc.scalar.mul(u(1, k), A1(k), w01[:, t:t + 1])
                        for k in range(CHUNK):
                            t = ch * CHUNK + k
                            nc.scalar.mul(u(2, k), B1(k), w11[:, t:t + 1])
                        for k in range(CHUNK):
                            t = ch * CHUNK + k
                            nc.vector.tensor_scalar(
                                out=ac(k), in0=A0(k), scalar1=w00[:, t:t + 1],
                                scalar2=None, op0=Alu.mult)
                        for k in range(CHUNK):
                            nc.vector.tensor_tensor(
                                out=u(1, k), in0=u(1, k), in1=u(2, k), op=Alu.add)
                        for k in range(CHUNK):
                            nc.vector.tensor_tensor(
                                out=ac(k), in0=ac(k), in1=u(0, k), op=Alu.add)
                        for k in range(CHUNK):
                            nc.vector.tensor_tensor(
                                out=ac(k), in0=ac(k), in1=u(1, k), op=Alu.add)
                    else:
                        # DVE only: 4 ts + 3 tt
                        mt = tpool.tile([P, 3 * CHUNK * C], F16, tag="mt")

                        def m(j, k):
                            return mt[:, (j * CHUNK + k) * C:
                                      (j * CHUNK + k + 1) * C]
                        for k in range(CHUNK):
                            t = ch * CHUNK + k
                            nc.vector.tensor_scalar(
                                out=ac(k), in0=A0(k), scalar1=w00[:, t:t + 1],
                                scalar2=None, op0=Alu.mult)
                        for k in range(CHUNK):
                            t = ch * CHUNK + k
                            nc.vector.tensor_scalar(
                                out=m(0, k), in0=B0(k), scalar1=w10[:, t:t + 1],
                                scalar2=None, op0=Alu.mult)
                        for k in range(CHUNK):
                            t = ch * CHUNK + k
                            nc.vector.tensor_scalar(
                                out=m(1, k), in0=A1(k), scalar1=w01[:, t:t + 1],
                                scalar2=None, op0=Alu.mult)
                        for k in range(CHUNK):
                            t = ch * CHUNK + k
                            nc.vector.tensor_scalar(
                                out=m(2, k), in0=B1(k), scalar1=w11[:, t:t + 1],
                                scalar2=None, op0=Alu.mult)
                        for k in range(CHUNK):
                            nc.vector.tensor_tensor(
                                out=ac(k), in0=ac(k), in1=m(0, k), op=Alu.add)
                        for k in range(CHUNK):
                            nc.vector.tensor_tensor(
                                out=m(1, k), in0=m(1, k), in1=m(2, k), op=Alu.add)
                        for k in range(CHUNK):
                            nc.vector.tensor_tensor(
                                out=ac(k), in0=ac(k), in1=m(1, k), op=Alu.add)
                    nc.sync.dma_start(
                        out=out[:, ch * CHUNK * C:(ch + 1) * CHUNK * C],
                        in_=afat[:, 0:CHUNK * C])

    nc.compile()
    return nc


def _prep_fgat(features):
    """Pair-interleaved fp16 + per-pixel-quantized int8 gather maps.

    Slot s covers rows (y0, y0+1) of one padded column x:
      fgat16[s] = [row0 fp16 | row1 fp16]   (2KB)
      fgat8[s]  = [row0 int8 | row1 int8]   (1KB), row r quantized by
                  S[y, x] = maxabs(p2[y, x, :]) / 127 (0 -> 1)
    Returns (fgat8 [NSLOT, 2C] int8, fgat16 [NSLOT, 2C] fp16, S [HP2, WP2]).
    """
    f = np.asarray(features, dtype=np.float32)
    p2 = np.zeros((HP2, WP2, C), dtype=np.float32)
    p2[2:130, 2:130, :] = f
    s = np.max(np.abs(p2), axis=2) / 127.0
    s[s == 0.0] = 1.0
    q = np.rint(p2 / s[:, :, None]).astype(np.int8)
    p16 = p2.astype(np.float16)

    def interleave(a):
        e = np.ascontiguousarray(
            a.reshape(NBLK_E, 2, WP2, C).transpose(0, 2, 1, 3)
        ).reshape(NSLOT_E, 2 * C)
        o = np.ascontiguousarray(
            a[1:131].reshape(NBLK_O, 2, WP2, C).transpose(0, 2, 1, 3)
        ).reshape(NBLK_O * WP2, 2 * C)
        return np.concatenate([e, o], axis=0)

    return interleave(q), interleave(p16), s.astype(np.float32)


def _prep_meta(boxes, s, npt8=NPT8):
    """Per-(box,point) gather slot index and blend weights.

    Mirrors the reference affine-grid math in float32:
      yf = BY*(0.5*bh-0.5) + (yc-1),  xf = BX*(0.5*bw-0.5) + (xc-1)
    with BY/BX the 7x7 [-1,1] grid; then y0=floor(yf), wy=yf-y0 (same for x).
    OOB corners are mapped to zero border rows/cols of the padded map, so the
    weights need no in-bounds masking.  For the first npt8 points (int8
    gathers) each corner weight is multiplied by that corner pixel's int8
    dequant scale.

    Returns (slot int32 [B,49], wts f32 [B, 4*49]).
    """
    b = np.asarray(boxes, dtype=np.float32)
    xc, yc, bw, bh = b[:, 0:1], b[:, 1:2], b[:, 2:3], b[:, 3:4]
    base = np.linspace(-1.0, 1.0, 7).astype(np.float32)
    BY = np.repeat(base, 7)[None, :]   # (1,49)
    BX = np.tile(base, 7)[None, :]
    yf = (BY * (np.float32(0.5) * bh - np.float32(0.5)) + (yc - 1)).astype(np.float32)
    xf = (BX * (np.float32(0.5) * bw - np.float32(0.5)) + (xc - 1)).astype(np.float32)
    y0 = np.floor(yf)
    x0 = np.floor(xf)
    wy = yf - y0
    wx = xf - x0
    wyc = np.float32(1.0) - wy
    wxc = np.float32(1.0) - wx
    pyA = np.clip(y0 + 2.0, 0.0, 130.0)
    px = np.clip(x0, -2.0, 128.0) + 2.0
    half = np.floor(pyA * 0.5)
    par = pyA - 2.0 * half
    slot = (par * NSLOT_E + half * WP2 + px).astype(np.int32)
    ri = pyA.astype(np.int32)
    ci = px.astype(np.int32)
    is8 = (np.arange(NPT) < npt8)[None, :]
    s00 = np.where(is8, s[ri, ci], np.float32(1.0))
    s10 = np.where(is8, s[ri + 1, ci], np.float32(1.0))
    s01 = np.where(is8, s[ri, ci + 1], np.float32(1.0))
    s11 = np.where(is8, s[ri + 1, ci + 1], np.float32(1.0))
    wts = np.concatenate(
        [wyc * wxc * s00, wy * wxc * s10, wyc * wx * s01, wy * wx * s11],
        axis=1).astype(np.float32)
    return slot, wts


def _wrap_idx(slot_core):
    """[P, 49] int32 slots -> [P, NCHUNK*IDXCOLS] int16 wrapped+replicated."""
    gi = np.zeros((P, NCHUNK * IDXCOLS), dtype=np.int16)
    for ch in range(NCHUNK):
        flat = slot_core[:, ch * CHUNK:(ch + 1) * CHUNK].T.reshape(-1)  # j*P+p
        cols = flat.reshape(IDXCOLS, 16)  # i//16 , i%16
        for k in range(8):
            gi[16 * k:16 * (k + 1), ch * IDXCOLS:(ch + 1) * IDXCOLS] = \
                cols.T.astype(np.int16)
    return gi


def _in_maps(features, boxes):
    fgat8, fgat16, s = _prep_fgat(features)
    slot, wts = _prep_meta(boxes, s)
    maps = []
    for k in range(NCORES):
        sl = slot[k * P:(k + 1) * P]
        maps.append({
            "fgat8": fgat8,
            "fgat16": fgat16,
            "gidx": _wrap_idx(sl),
            "wts": np.ascontiguousarray(wts[k * P:(k + 1) * P]),
        })
    return maps


def kernel(features, boxes, image_height=128, image_width=128):
    from concourse.bass_utils import run_bass_kernel_spmd

    if "nc" not in _STATE:
        _STATE["nc"] = _build_nc()
    nc = _STATE["nc"]

    in_maps = _in_maps(features, boxes)
    res = run_bass_kernel_spmd(
        nc, in_maps, core_ids=list(range(NCORES)),
        trace=_STATE.get("trace", False),
    )
    _STATE["last"] = res
    out = np.concatenate(
        [res.results[k]["out"].reshape(P, 7, 7, C).astype(np.float32)
         for k in range(NCORES)],
        axis=0,
    )
    return out
